# revision 1
# baseline (speedup 1.0000x reference)
"""CAM (channel attention) module kernel for Trainium2, 8-core data-parallel.

Reference computation (per sample b):
    q = conv2d(x, Wq, stride2, 2x2) -> [C, 4096]
    k = conv2d(x, Wk, stride2, 2x2) -> [C, 4096]
    v = conv2d(x, Wv, 1x1)          -> [C, 16384]
    E = q @ k^T                      [C, C]
    att = softmax(rowmax(E) - E)   (== softmin over rows)
    out = att @ v -> [C, H, W]

Kernel strategy (one sample per NeuronCore, B=8 over 8 cores):
  - The softmax here is extremely peaked (energy entries span +-200), so
    energy errors are amplified exponentially: q/k need ~18+ mantissa
    bits, which rules out bf16 and single-pass fp32r (12-bit) for the
    convs. Native fp32 matmul costs 2 half-rate passes (4 cyc/row).
  - Measured on HW: 4-byte moving operands (fp32 AND fp32r) stream at
    2 cyc/row; bf16 streams at 1 cyc/row. So the cheapest precise
    scheme is split-bf16: x = xh + xl with xh = bf16(x), xl =
    bf16(x - xh) (~16-bit combined); same for the conv weights.
    conv = Wh@xh + Wh@xl + Wl@xh: 3 full-rate bf16 passes (3 cyc/row
    vs fp32's 4) with ~6.5e-4 worst-case output impact.
  - conv produces q in [c, n] layout via strided im2col APs from the
    resident xr/xl2 tiles (4 accumulating taps x 3 passes per band),
    then PE-transposes to [n, c] chunks for the energy contraction.
  - energy e = q k^T in native fp32 (exact; N=128 makes fp32r slow
    there anyway), accumulated over 32 chunk matmuls in one PSUM bank.
  - softmax via one DVE row-min + one ScalarE exp (bias=rowmin,
    scale=-1) with fused accumulated row-sum.
  - out = att @ (Wv x + bv) == (att Wv) @ x + (att bv) 1^T: computes
    M^T = Wv^T att^T on PE ([128,128]), splits M the same way, and
    runs out = Mh@xh + Mh@xl + Ml@xh against the resident split-x
    tiles (3 bf16 passes, ~1e-5 error; reuses the conv's xh/xl).
"""

import numpy as np

B, C, H, W = 8, 128, 128, 128
HW = H * W           # 16384
N_CORES = 8
NB = 8               # number of H-bands (16 input rows each)
BAND = HW // NB      # 2048 x columns per band
QN = (H // 2) * (W // 2)  # 4096 conv output positions
QCHUNK = QN // NB    # 512 conv outputs per band

_CACHE = {}


def _build_program(with_qk_bias: bool, with_v_bias: bool):
    import concourse.tile as tile
    from concourse import bacc, mybir
    from concourse.masks import make_identity

    f32 = mybir.dt.float32
    bf16 = mybir.dt.bfloat16
    Ident = mybir.ActivationFunctionType.Identity
    CopyF = mybir.ActivationFunctionType.Copy
    nc = bacc.Bacc(
        "TRN2", target_bir_lowering=False, debug=False, num_devices=N_CORES)

    x_d = nc.declare_dram_parameter("x", [C, HW], f32, isOutput=False)
    wqk_d = nc.declare_dram_parameter("wqk", [C, 8 * C], f32, isOutput=False)
    wv_d = nc.declare_dram_parameter("wv", [C, C], f32, isOutput=False)
    if with_qk_bias:
        bq_d = nc.declare_dram_parameter("bq", [C, 1], f32, isOutput=False)
        bk_d = nc.declare_dram_parameter("bk", [C, 1], f32, isOutput=False)
    if with_v_bias:
        bv_d = nc.declare_dram_parameter("bv", [C, 1], f32, isOutput=False)
    out_d = nc.declare_dram_parameter("out", [C, HW], f32, isOutput=True)

    with tile.TileContext(nc) as tc:
        with (
            tc.tile_pool(name="const", bufs=1) as const,
            tc.tile_pool(name="xstage", bufs=3) as xstage,
            tc.tile_pool(name="xrp", bufs=1) as xrp,
            tc.tile_pool(name="xl2p", bufs=3) as xl2p,
            tc.tile_pool(name="qkT", bufs=1) as qkT,
            tc.tile_pool(name="stage", bufs=3) as stage,
            tc.tile_pool(name="oout", bufs=3) as oout,
            tc.tile_pool(name="small", bufs=2) as small,
            tc.tile_pool(name="pacc", bufs=4, space="PSUM") as pacc,
            tc.tile_pool(name="ptp", bufs=2, space="PSUM") as ptp,
            tc.tile_pool(name="psm", bufs=1, space="PSUM") as psm,
        ):
            ident = const.tile([128, 128], f32, tag="ident")
            make_identity(nc, ident)

            # Startup: per-core DMA bandwidth (~350GB/s) is ONE shared
            # resource; concurrent queues just split it and delay the
            # critical band-0 input. Serialize ALL input loads on the sync
            # queue in consumption order so each transfer gets full
            # bandwidth: x0 first, then the conv weights, then the
            # remaining bands (each lands well before its conv slot).
            x_sb = []
            xh_sb = [xrp.tile([C, BAND], bf16, tag=f"xh{j}", name=f"xh{j}")
                     for j in range(NB)]
            xl_sb = [xrp.tile([C, BAND], bf16, tag=f"xl{j}", name=f"xl{j}")
                     for j in range(NB)]
            x0 = xstage.tile([C, BAND], f32, tag="x")
            nc.sync.dma_start(out=x0, in_=x_d[:, 0:BAND])
            x_sb.append(x0)
            wqk_sb = const.tile([C, 8 * C], f32, tag="wqk")
            nc.sync.dma_start(out=wqk_sb, in_=wqk_d[:, :])
            wqT_sb = wqk_sb[:, 0:4 * C]
            wkT_sb = wqk_sb[:, 4 * C:8 * C]
            if with_qk_bias:
                bq_sb = const.tile([C, 1], f32, tag="bq")
                nc.sync.dma_start(out=bq_sb, in_=bq_d[:, :])
                bk_sb = const.tile([C, 1], f32, tag="bk")
                nc.sync.dma_start(out=bk_sb, in_=bk_d[:, :])
            for j in range(1, NB):
                t = xstage.tile([C, BAND], f32, tag="x", name=f"x{j}")
                nc.sync.dma_start(out=t, in_=x_d[:, j * BAND:(j + 1) * BAND])
                x_sb.append(t)
            wv_sb = const.tile([C, C], f32, tag="wv")
            nc.sync.dma_start(out=wv_sb, in_=wv_d[:, :])
            if with_v_bias:
                bv_sb = const.tile([C, 1], f32, tag="bv")
                nc.sync.dma_start(out=bv_sb, in_=bv_d[:, :])

            # split the conv weights into bf16 hi/lo; keep DVE free for the
            # x band-0 quarter splits (hi cast on ScalarE, lo sub on GpSimd)
            def split_w(w_f32, name):
                hi = const.tile([C, 4 * C], bf16, tag=f"{name}hi")
                nc.scalar.activation(out=hi, in_=w_f32, func=CopyF,
                                     bias=0.0, scale=1.0)
                lo = const.tile([C, 4 * C], bf16, tag=f"{name}lo")
                nc.gpsimd.tensor_tensor(
                    out=lo, in0=w_f32, in1=hi,
                    op=mybir.AluOpType.subtract)
                return hi, lo

            wqh, wql = split_w(wqT_sb, "wq")
            wkh, wkl = split_w(wkT_sb, "wk")

            qT = [qkT.tile([128, QCHUNK], f32, tag=f"qT{j}", name=f"qT{j}")
                  for j in range(NB)]
            kT = [qkT.tile([128, QCHUNK], f32, tag=f"kT{j}", name=f"kT{j}")
                  for j in range(NB)]

            def conv_band(j, wh, wl, xr_v, xl_v, bias_sb):
                """12 accumulating matmuls -> PSUM [128, 512] (q for band j),
                returns the psum tile."""
                acc = pacc.tile([128, QCHUNK], f32, tag="acc")
                n_mm = 0
                for ab in range(4):
                    a, bb = ab // 2, ab % 2
                    for lhsT, rhs in (
                        (wh[:, ab * C:(ab + 1) * C], xr_v[:, :, a, :, bb]),
                        (wh[:, ab * C:(ab + 1) * C], xl_v[:, :, a, :, bb]),
                        (wl[:, ab * C:(ab + 1) * C], xr_v[:, :, a, :, bb]),
                    ):
                        nc.tensor.matmul(acc, lhsT=lhsT, rhs=rhs,
                                         start=(n_mm == 0), stop=(n_mm == 11))
                        n_mm += 1
                return acc

            def emit_transposes(j, qc, kc):
                for T_out, src in ((qT[j], qc), (kT[j], kc)):
                    tp = ptp.tile([128, QCHUNK], f32, tag="tp")
                    for t in range(4):
                        nc.tensor.transpose(
                            tp[:, t * 128:(t + 1) * 128],
                            src[:, t * 128:(t + 1) * 128], ident)
                    nc.scalar.activation(out=T_out, in_=tp, func=CopyF,
                                         bias=0.0, scale=1.0)

            # energy accumulator lives across the whole conv phase: energy
            # chunk matmuls are interleaved into the conv stream so their
            # weight loads hide under conv matmuls and the PE never sits in
            # a low-duty phase (which would re-throttle the HAM clock gate).
            E = psm.tile([128, 128], f32, tag="E")
            e_idx = [0]

            def emit_energy(j):
                for t in range(4):
                    nc.tensor.matmul(
                        E,
                        lhsT=qT[j][:, t * 128:(t + 1) * 128],
                        rhs=kT[j][:, t * 128:(t + 1) * 128],
                        start=(e_idx[0] == 0), stop=(e_idx[0] == NB * 4 - 1))
                    e_idx[0] += 1

            def split_band(j):
                xh_t, xl_t = xh_sb[j], xl_sb[j]
                nc.vector.tensor_copy(xh_t, x_sb[j])
                nc.vector.tensor_tensor(
                    out=xl_t, in0=x_sb[j], in1=xh_t,
                    op=mybir.AluOpType.subtract)

            pend = None
            for j in range(NB):
                split_band(j)
                xh_t, xl_t = xh_sb[j], xl_sb[j]
                xr_v = xh_t[:].rearrange(
                    "p (i a w b) -> p i a w b", i=8, a=2, w=64, b=2)
                xl_v = xl_t[:].rearrange(
                    "p (i a w b) -> p i a w b", i=8, a=2, w=64, b=2)
                acc_q = conv_band(j, wqh, wql, xr_v, xl_v, None)
                acc_k = conv_band(j, wkh, wkl, xr_v, xl_v, None)

                qc = stage.tile([128, QCHUNK], f32, tag="qchunk")
                kc = stage.tile([128, QCHUNK], f32, tag="kchunk")
                if with_qk_bias:
                    nc.scalar.activation(out=qc, in_=acc_q, func=Ident,
                                         bias=bq_sb[:, 0:1], scale=1.0)
                    nc.scalar.activation(out=kc, in_=acc_k, func=Ident,
                                         bias=bk_sb[:, 0:1], scale=1.0)
                else:
                    nc.scalar.activation(out=qc, in_=acc_q, func=CopyF,
                                         bias=0.0, scale=1.0)
                    nc.scalar.activation(out=kc, in_=acc_k, func=CopyF,
                                         bias=0.0, scale=1.0)
                # transposes + energy one band behind
                if pend is not None:
                    emit_transposes(*pend)
                    emit_energy(pend[0])
                pend = (j, qc, kc)
            emit_transposes(*pend)
            emit_energy(pend[0])

            # keep the PE busy through the softmax serial chain so the HAM
            # clock gate doesn't re-throttle before the output matmuls
            # (results unused; inputs are long since ready)
            for dw in range(28):
                scratch = pacc.tile([128, 256], f32, tag="acc",
                                    name=f"warm{dw}")
                nc.tensor.matmul(
                    scratch, lhsT=wqh[:, 0:128],
                    rhs=xh_sb[0][:, 0:256],
                    start=True, stop=True)

            # softmin over rows: att = exp(rowmin - E) / Z
            mmin = small.tile([128, 1], f32, tag="mmin")
            nc.vector.tensor_reduce(
                out=mmin, in_=E, axis=mybir.AxisListType.X,
                op=mybir.AluOpType.min)
            w_sb = small.tile([128, 128], f32, tag="w")
            zsum = small.tile([128, 1], f32, tag="z")
            nc.scalar.activation(
                out=w_sb, in_=E, func=mybir.ActivationFunctionType.Exp,
                bias=mmin[:, 0:1], scale=-1.0, accum_out=zsum[:, 0:1])
            rz = small.tile([128, 1], f32, tag="rz")
            nc.vector.reciprocal(rz, zsum)
            att = small.tile([128, 128], f32, tag="att")
            nc.vector.tensor_scalar_mul(att, w_sb, rz[:, 0:1])

            attT_p = psm.tile([128, 128], f32, tag="s2")
            nc.tensor.transpose(attT_p, att, ident)
            attT = small.tile([128, 128], f32, tag="attT")
            nc.vector.tensor_copy(attT, attT_p)

            # M^T[c2, c] = sum_d Wv[d, c2] attT[d, c], split into bf16 hi/lo
            MT_p = psm.tile([128, 128], f32, tag="s2")
            nc.tensor.matmul(MT_p, lhsT=wv_sb, rhs=attT, start=True, stop=True)
            Mh = small.tile([128, 128], bf16, tag="Mh")
            nc.vector.tensor_copy(Mh, MT_p)
            Ml = small.tile([128, 128], bf16, tag="Ml")
            nc.vector.tensor_tensor(
                out=Ml, in0=MT_p, in1=Mh, op=mybir.AluOpType.subtract)

            if with_v_bias:
                abv_p = psm.tile([128, 1], f32, tag="s2")
                nc.tensor.matmul(abv_p, lhsT=attT, rhs=bv_sb[:, 0:1],
                                 start=True, stop=True)
                abv = small.tile([128, 1], f32, tag="abv")
                nc.vector.tensor_copy(abv, abv_p)

            # out[c, n] = sum_c2 M[c, c2] x[c2, n] (+ abv[c]) via bf16 split.
            # Stationary-major order within each band: one LDW for Mh across
            # 8 matmuls, one for Ml across 4, with 4 PSUM accumulators in
            # flight.
            out_dma_engines = [nc.sync, nc.gpsimd, nc.scalar]
            for j in range(NB):
                o_band = oout.tile([128, BAND], f32, tag="oband")
                o_ps = [pacc.tile([128, 512], f32, tag="acc",
                                  name=f"ops{j}_{s}")
                        for s in range(4)]
                for s in range(4):
                    nc.tensor.matmul(
                        o_ps[s], lhsT=Mh,
                        rhs=xh_sb[j][:, s * 512:(s + 1) * 512],
                        start=True, stop=False)
                for s in range(4):
                    nc.tensor.matmul(
                        o_ps[s], lhsT=Mh,
                        rhs=xl_sb[j][:, s * 512:(s + 1) * 512],
                        start=False, stop=False)
                for s in range(4):
                    nc.tensor.matmul(
                        o_ps[s], lhsT=Ml,
                        rhs=xh_sb[j][:, s * 512:(s + 1) * 512],
                        start=False, stop=True)
                for s in range(4):
                    dst = o_band[:, s * 512:(s + 1) * 512]
                    if with_v_bias:
                        nc.scalar.activation(
                            out=dst, in_=o_ps[s], func=Ident,
                            bias=abv[:, 0:1], scale=1.0)
                    elif s % 2 == 0:
                        nc.vector.tensor_copy(dst, o_ps[s])
                    else:
                        nc.scalar.activation(out=dst, in_=o_ps[s], func=CopyF,
                                             bias=0.0, scale=1.0)
                pieces = 2 if j == NB - 1 else 1
                psz = BAND // pieces
                for h in range(pieces):
                    off = j * BAND + h * psz
                    out_dma_engines[(j + h) % 3].dma_start(
                        out=out_d[:, off:off + psz],
                        in_=o_band[:, h * psz:(h + 1) * psz])

    nc.compile()
    return nc


def kernel(x, Wq, bq, Wk, bk, Wv, bv):
    from concourse.bass_utils import run_bass_kernel_spmd

    x = np.ascontiguousarray(np.asarray(x, dtype=np.float32))
    Wq = np.asarray(Wq, dtype=np.float32)
    Wk = np.asarray(Wk, dtype=np.float32)
    Wv = np.asarray(Wv, dtype=np.float32)
    bq = np.asarray(bq, dtype=np.float32)
    bk = np.asarray(bk, dtype=np.float32)
    bv = np.asarray(bv, dtype=np.float32)

    with_qk_bias = bool(np.any(bq) or np.any(bk))
    with_v_bias = bool(np.any(bv))

    key = (with_qk_bias, with_v_bias)
    if key not in _CACHE:
        _CACHE[key] = _build_program(with_qk_bias, with_v_bias)
    nc = _CACHE[key]

    # weight layout prep: wT[cin, ab*128 + c] = W[c, cin, a, b];
    # q and k weights packed into one tensor for a single early DMA
    wqT = Wq.transpose(1, 2, 3, 0).reshape(C, 4 * C)
    wkT = Wk.transpose(1, 2, 3, 0).reshape(C, 4 * C)
    wqk = np.ascontiguousarray(np.concatenate([wqT, wkT], axis=1))
    wv = np.ascontiguousarray(Wv.reshape(C, C))

    in_maps = []
    for b in range(B):
        m = {
            "x": np.ascontiguousarray(x[b].reshape(C, HW)),
            "wqk": wqk,
            "wv": wv,
        }
        if with_qk_bias:
            m["bq"] = np.ascontiguousarray(bq.reshape(C, 1))
            m["bk"] = np.ascontiguousarray(bk.reshape(C, 1))
        if with_v_bias:
            m["bv"] = np.ascontiguousarray(bv.reshape(C, 1))
        in_maps.append(m)

    res = run_bass_kernel_spmd(nc, in_maps, list(range(N_CORES)))
    out = np.stack([res.results[i]["out"] for i in range(N_CORES)])
    return out.reshape(B, C, H, W).astype(np.float32)



# revision 4
# speedup vs baseline: 1.1148x; 1.1148x over previous
"""CAM (channel attention) module kernel for Trainium2, 8-core data-parallel.

Reference computation (per sample b):
    q = conv2d(x, Wq, stride2, 2x2) -> [C, 4096]
    k = conv2d(x, Wk, stride2, 2x2) -> [C, 4096]
    v = conv2d(x, Wv, 1x1)          -> [C, 16384]
    E = q @ k^T                      [C, C]
    att = softmax(rowmax(E) - E)   (== softmin over rows)
    out = att @ v -> [C, H, W]

Kernel strategy (one sample per NeuronCore, B=8 over 8 cores):
  - Precision: the softmax is extremely peaked (energy spans +-200), so
    q/k need ~16 mantissa bits -> split-bf16 conv (3 full-rate bf16
    passes: Wh@xh + Wl@xh + Wh@xl). The hi/lo split of x AND the conv
    weights is done HOST-side: xh+xl bf16 is the same DMA bytes as fp32
    x, and it removes all on-chip split work plus the startup split
    latency chain.
  - Convs are x-stationary producing qT/kT DIRECTLY in [n, c] layout
    (lhsT = im2col AP of the resident xh/xl chunk, moving = conv
    weights with q and k fused on the free axis -> [cin, 256] per tap).
    This kills the PE transposes of the W-stationary formulation.
    32 chunks of 128 conv outputs; per chunk 12 bf16 matmuls of 256
    rows + 1 fp32 energy matmul accumulating E in one PSUM bank.
  - softmax via one DVE row-min + one ScalarE exp (bias=rowmin,
    scale=-1) with fused row-sum; 1/Z is folded into att BEFORE the
    M = att @ Wv trick, so out needs no per-row post-scale.
  - out = M @ x in ONE bf16 pass (Mh @ xh): linear path, no exp
    amplification; adds ~2e-3 rel err (gate is 2e-2). Output phase is
    DMA-bound (8MB at ~350GB/s), PE does only 32 matmuls.
  - PE p-state: the tensor engine ramps over ~3us after idle. Dummy
    warm-up matmuls run from t~0 through the DMA/boot window so the
    first real conv starts at full clock.
"""

import numpy as np

B, C, H, W = 8, 128, 128, 128
HW = H * W             # 16384
N_CORES = 8
NCH = 32               # conv chunks; each covers 512 x-columns -> 128 q rows
WARM_N = 70            # PE warm-up matmuls covering the boot+DMA window

_CACHE = {}


def _build_program(with_qk_bias: bool, with_v_bias: bool):
    import concourse.tile as tile
    from concourse import bacc, mybir
    from concourse.masks import make_identity

    f32 = mybir.dt.float32
    bf16 = mybir.dt.bfloat16
    Ident = mybir.ActivationFunctionType.Identity
    CopyF = mybir.ActivationFunctionType.Copy
    nc = bacc.Bacc(
        "TRN2", target_bir_lowering=False, debug=False, num_devices=N_CORES)

    xrh_d = nc.declare_dram_parameter("xrh", [C, HW], bf16, isOutput=False)
    xrl_d = nc.declare_dram_parameter("xrl", [C, HW], bf16, isOutput=False)
    xnh_d = nc.declare_dram_parameter("xnh", [C, HW], bf16, isOutput=False)
    wqkh_d = nc.declare_dram_parameter("wqkh", [C, 8 * C], bf16, isOutput=False)
    wqkl_d = nc.declare_dram_parameter("wqkl", [C, 8 * C], bf16, isOutput=False)
    wv_d = nc.declare_dram_parameter("wv", [C, C], f32, isOutput=False)
    if with_qk_bias:
        bqk_d = nc.declare_dram_parameter("bqk", [1, 2 * C], f32, isOutput=False)
    if with_v_bias:
        bv_d = nc.declare_dram_parameter("bv", [C, 1], f32, isOutput=False)
    out_d = nc.declare_dram_parameter("out", [C, HW], f32, isOutput=True)

    with tile.TileContext(nc) as tc:
        with (
            tc.tile_pool(name="const", bufs=1) as const,
            tc.tile_pool(name="xp", bufs=1) as xp,
            tc.tile_pool(name="qkstage", bufs=3) as qkstage,
            tc.tile_pool(name="oout", bufs=3) as oout,
            tc.tile_pool(name="small", bufs=2) as small,
            tc.tile_pool(name="pconv", bufs=2, space="PSUM") as pconv,
            tc.tile_pool(name="psm", bufs=1, space="PSUM") as psm,
            tc.tile_pool(name="pacc", bufs=4, space="PSUM") as pacc,
        ):
            # ---- PE warm-up: keep the tensor engine busy (and p-state
            # ramped) through the fixed boot + initial DMA window so real
            # convs start at full clock. Garbage in, garbage to scratch.
            warm = const.tile([128, 256], bf16, tag="warm")
            nc.gpsimd.memset(warm[:], 0.0)
            for i in range(WARM_N):
                wt = pacc.tile([128, 512], f32, tag="acc", name=f"warm{i}")
                nc.tensor.matmul(wt[:, 0:256], lhsT=warm[:, 0:128], rhs=warm[:],
                                 start=True, stop=True)

            # ---- input DMAs, consumption order, all on the sync queue.
            # First conv chunk needs wqkh + xh cols 0:512 only.
            wqkh_sb = const.tile([C, 8 * C], bf16, tag="wqkh")
            wqkl_sb = const.tile([C, 8 * C], bf16, tag="wqkl")
            # x parts: chunks 0-3 as single-chunk tiles (fine-grained
            # startup), then 1024-col pair tiles.
            xrh_parts, xrl_parts = [], []
            for t, parts in ((0, xrh_parts), (1, xrl_parts)):
                for i in range(4):
                    parts.append(xp.tile([C, 512], bf16, tag=f"x{t}q{i}",
                                         name=f"x{t}q{i}"))
                for i in range(14):
                    parts.append(xp.tile([C, 1024], bf16, tag=f"x{t}p{i}",
                                         name=f"x{t}p{i}"))
            xn_bands = [xp.tile([C, 2048], bf16, tag=f"xn{j}", name=f"xn{j}")
                        for j in range(8)]

            def x_part(parts, ci):
                if ci < 4:
                    return parts[ci], 0
                return parts[4 + (ci - 4) // 2], 512 * ((ci - 4) % 2)

            nc.sync.dma_start(out=wqkh_sb, in_=wqkh_d[:, :])
            nc.sync.dma_start(out=xrh_parts[0], in_=xrh_d[:, 0:512])
            nc.sync.dma_start(out=wqkl_sb, in_=wqkl_d[:, :])
            nc.sync.dma_start(out=xrl_parts[0], in_=xrl_d[:, 0:512])
            for i in range(1, 4):
                nc.sync.dma_start(out=xrh_parts[i], in_=xrh_d[:, 512 * i:512 * (i + 1)])
                nc.sync.dma_start(out=xrl_parts[i], in_=xrl_d[:, 512 * i:512 * (i + 1)])
            for i in range(14):
                lo, hi = 2048 + 1024 * i, 2048 + 1024 * (i + 1)
                nc.sync.dma_start(out=xrh_parts[4 + i], in_=xrh_d[:, lo:hi])
                nc.sync.dma_start(out=xrl_parts[4 + i], in_=xrl_d[:, lo:hi])
            wv_sb = const.tile([C, C], f32, tag="wv")
            nc.sync.dma_start(out=wv_sb, in_=wv_d[:, :])
            for j in range(8):
                nc.sync.dma_start(out=xn_bands[j],
                                  in_=xnh_d[:, j * 2048:(j + 1) * 2048])
            if with_qk_bias:
                bqk_sb = const.tile([1, 2 * C], f32, tag="bqk")
                nc.sync.dma_start(out=bqk_sb, in_=bqk_d[:, :])
                ones1 = const.tile([1, 128], f32, tag="ones1")
                nc.gpsimd.memset(ones1[:], 1.0)
            if with_v_bias:
                bv_sb = const.tile([C, 1], f32, tag="bv")
                nc.sync.dma_start(out=bv_sb, in_=bv_d[:, :])

            ident = const.tile([128, 128], f32, tag="ident")
            make_identity(nc, ident)

            # ---- conv + energy: per chunk, qT|kT [128n, 256] via 12 bf16
            # matmuls (x chunk stationary, fused q|k weights moving), then
            # one fp32 matmul accumulating E. The E matmul for chunk ci-1
            # is emitted during chunk ci so its ScalarE PSUM->SBUF copy has
            # a full chunk of slack.
            E = psm.tile([128, 128], f32, tag="E")
            n_mm = 13 if with_qk_bias else 12
            pend = None
            for ci in range(NCH):
                xh_t, off = x_part(xrh_parts, ci)
                xl_t, _ = x_part(xrl_parts, ci)
                ps = pconv.tile([128, 256], f32, tag="qk")
                idx = 0
                for ab in range(4):
                    xsl = slice(off + ab * 128, off + (ab + 1) * 128)
                    wsl = slice(ab * 256, (ab + 1) * 256)
                    for lhsT, rhs in (
                        (xh_t[:, xsl], wqkh_sb[:, wsl]),
                        (xh_t[:, xsl], wqkl_sb[:, wsl]),
                        (xl_t[:, xsl], wqkh_sb[:, wsl]),
                    ):
                        nc.tensor.matmul(ps, lhsT=lhsT, rhs=rhs,
                                         start=(idx == 0), stop=(idx == n_mm - 1))
                        idx += 1
                if with_qk_bias:
                    nc.tensor.matmul(ps, lhsT=ones1[:, :], rhs=bqk_sb[:, :],
                                     start=False, stop=True,
                                     skip_group_check=True)
                qk_sb = qkstage.tile([128, 256], f32, tag="qks",
                                     name=f"qks{ci}")
                nc.scalar.activation(out=qk_sb, in_=ps, func=CopyF,
                                     bias=0.0, scale=1.0)
                if pend is not None:
                    nc.tensor.matmul(E, lhsT=pend[:, 0:128],
                                     rhs=pend[:, 128:256],
                                     start=(ci == 1), stop=False)
                pend = qk_sb
            nc.tensor.matmul(E, lhsT=pend[:, 0:128], rhs=pend[:, 128:256],
                             start=False, stop=True)

            # ---- softmin over rows: att = exp(rowmin - E) / Z, with 1/Z
            # folded into att before the M trick.
            mmin = small.tile([128, 1], f32, tag="mmin")
            nc.vector.tensor_reduce(
                out=mmin, in_=E, axis=mybir.AxisListType.X,
                op=mybir.AluOpType.min)
            w_sb = small.tile([128, 128], f32, tag="w")
            zsum = small.tile([128, 1], f32, tag="z")
            nc.scalar.activation(
                out=w_sb, in_=E, func=mybir.ActivationFunctionType.Exp,
                bias=mmin[:, 0:1], scale=-1.0, accum_out=zsum[:, 0:1])
            rz = small.tile([128, 1], f32, tag="rz")
            nc.vector.reciprocal(rz, zsum)
            att = small.tile([128, 128], f32, tag="att")
            nc.vector.tensor_scalar_mul(att, w_sb, rz[:, 0:1])

            attT_p = psm.tile([128, 128], f32, tag="s2")
            nc.tensor.transpose(attT_p, att, ident)
            attT = small.tile([128, 128], f32, tag="attT")
            nc.vector.tensor_copy(attT, attT_p)

            # M^T[i, c] = sum_o Wv[o, i] attT[o, c]; bf16 is enough for the
            # single-pass output matmul.
            MT_p = psm.tile([128, 128], f32, tag="s2")
            nc.tensor.matmul(MT_p, lhsT=wv_sb, rhs=attT, start=True, stop=True)
            Mh = small.tile([128, 128], bf16, tag="Mh")
            nc.vector.tensor_copy(Mh, MT_p)

            if with_v_bias:
                abv_p = psm.tile([128, 1], f32, tag="s2")
                nc.tensor.matmul(abv_p, lhsT=attT, rhs=bv_sb[:, 0:1],
                                 start=True, stop=True)
                abv = small.tile([128, 1], f32, tag="abv")
                nc.vector.tensor_copy(abv, abv_p)

            # ---- out[c, n] = Mh @ xh, one bf16 pass; phase is DMA-bound
            # so copies alternate scalar/vector and bands rotate queues.
            out_dma_engines = [nc.sync, nc.gpsimd, nc.scalar]
            for j in range(8):
                o_band = oout.tile([128, 2048], f32, tag="oband")
                for s in range(4):
                    ci = j * 4 + s
                    o_ps = pacc.tile([128, 512], f32, tag="acc",
                                     name=f"ops{ci}")
                    nc.tensor.matmul(o_ps, lhsT=Mh[:, :],
                                     rhs=xn_bands[j][:, s * 512:(s + 1) * 512],
                                     start=True, stop=True)
                    dst = o_band[:, s * 512:(s + 1) * 512]
                    if with_v_bias:
                        if s % 2 == 0:
                            nc.scalar.activation(out=dst, in_=o_ps, func=Ident,
                                                 bias=abv[:, 0:1], scale=1.0)
                        else:
                            nc.vector.tensor_scalar_add(dst, o_ps, abv[:, 0:1])
                    else:
                        if s % 2 == 0:
                            nc.scalar.activation(out=dst, in_=o_ps, func=CopyF,
                                                 bias=0.0, scale=1.0)
                        else:
                            nc.vector.tensor_copy(dst, o_ps)
                pieces = 2 if j == 7 else 1
                psz = 2048 // pieces
                for h in range(pieces):
                    off = j * 2048 + h * psz
                    out_dma_engines[(j + h) % 3].dma_start(
                        out=out_d[:, off:off + psz],
                        in_=o_band[:, h * psz:(h + 1) * psz])

    nc.compile()
    return nc


def kernel(x, Wq, bq, Wk, bk, Wv, bv):
    import ml_dtypes
    from concourse.bass_utils import run_bass_kernel_spmd

    bf16 = ml_dtypes.bfloat16
    x = np.ascontiguousarray(np.asarray(x, dtype=np.float32))
    Wq = np.asarray(Wq, dtype=np.float32)
    Wk = np.asarray(Wk, dtype=np.float32)
    Wv = np.asarray(Wv, dtype=np.float32)
    bq = np.asarray(bq, dtype=np.float32)
    bk = np.asarray(bk, dtype=np.float32)
    bv = np.asarray(bv, dtype=np.float32)

    with_qk_bias = bool(np.any(bq) or np.any(bk))
    with_v_bias = bool(np.any(bv))

    key = (with_qk_bias, with_v_bias)
    if key not in _CACHE:
        _CACHE[key] = _build_program(with_qk_bias, with_v_bias)
    nc = _CACHE[key]

    # weight layout: wcat[cin, ab*256 + {0:128 -> q, 128:256 -> k} cout]
    wq_t = Wq.transpose(1, 2, 3, 0).reshape(C, 4, 1, C)
    wk_t = Wk.transpose(1, 2, 3, 0).reshape(C, 4, 1, C)
    wcat = np.concatenate([wq_t, wk_t], axis=2).reshape(C, 8 * C)
    wqkh = wcat.astype(bf16)
    wqkl = (wcat - wqkh.astype(np.float32)).astype(bf16)
    wqkh = np.ascontiguousarray(wqkh)
    wqkl = np.ascontiguousarray(wqkl)
    wv = np.ascontiguousarray(Wv.reshape(C, C))

    # conv layout: xr2[c, ci, a, b, di, w] = x[c, 4ci+2di+a, 2w+b]
    # (stride-2 2x2 taps partition x exactly; host-side "im2col")
    xr2 = x.reshape(B, C, 32, 2, 2, 64, 2).transpose(
        0, 1, 2, 4, 6, 3, 5).reshape(B, C, HW)
    xrh = xr2.astype(bf16)
    xrl = (xr2 - xrh.astype(np.float32)).astype(bf16)
    xnh = x.reshape(B, C, HW).astype(bf16)

    in_maps = []
    for b in range(B):
        m = {
            "xrh": np.ascontiguousarray(xrh[b]),
            "xrl": np.ascontiguousarray(xrl[b]),
            "xnh": np.ascontiguousarray(xnh[b]),
            "wqkh": wqkh,
            "wqkl": wqkl,
            "wv": wv,
        }
        if with_qk_bias:
            m["bqk"] = np.ascontiguousarray(
                np.concatenate([bq, bk]).reshape(1, 2 * C))
        if with_v_bias:
            m["bv"] = np.ascontiguousarray(bv.reshape(C, 1))
        in_maps.append(m)

    res = run_bass_kernel_spmd(nc, in_maps, list(range(N_CORES)))
    out = np.stack([res.results[i]["out"] for i in range(N_CORES)])
    return out.reshape(B, C, H, W).astype(np.float32)
